# revision 1
# baseline (speedup 1.0000x reference)
"""DMI loss kernel for Trainium2 (8 NeuronCores, data-parallel over batch).

reference:
    preds  = [x, 1-x]  [b, 2, hw]
    labels = [y, 1-y]  [b, 2, hw]
    mat    = preds @ labels.T          (per-sample 2x2)
    loss   = mean(-log(|det(mat)| + 1e-3))

Per sample only three reductions over hw are needed:
    S_x = sum(x), S_y = sum(y), S_xy = sum(x*y)
since det(mat) == hw*S_xy - S_x*S_y (exact algebraic identity).

Sharding: batch 64 -> 8 cores x 8 samples. Each core reduces its 8 samples to
per-partition partial sums on-device; the det/log/mean epilogue runs on host
in float64.

Device pipeline per core (memory-bound, ~404 GB/s/core measured):
  DMA : one 1 MiB HWDGE transfer per tensor per sample, all on the sync-engine
        ring so completion order matches sample order (x0,y0,x1,y1,...).
        The LAST sample is split into column chunks so the end-of-stream
        compute tail is one chunk, not a whole sample.
  DVE : tensor_mul + tensor_reduce (free axis)  -> S_xy per partition
  ACT : activation(Copy, accum_out)             -> S_x, S_y per partition
  out : stats tiles [128, NCOL] DMA'd back; host sums partitions in fp64.
"""

import sys

for _p in ("/opt/trn_rl_repo",):
    if _p not in sys.path:
        sys.path.append(_p)

import numpy as np
from contextlib import ExitStack

import concourse.bass as bass
import concourse.tile as tile
from concourse import bacc, mybir
from concourse.bass_utils import run_bass_kernel_spmd

N_CORES = 8
B = 64
H = W = 512
HW = H * W
S = B // N_CORES      # samples per core
P = 128               # SBUF partitions
F = HW // P           # free dim per partition
TAIL_SPLITS = (1024, 1536, 2048)  # uneven chunk boundaries for the last
                                  # sample: the final chunks are small so the
                                  # end-of-stream DVE tail is short
TAIL_CHUNKS = len(TAIL_SPLITS)
N_PAIRS = 0           # 2MiB sample-pair transfers disabled: the cost-model
                      # shows they delay pipeline ramp more than they save
N_SINGLE = S - 2 * N_PAIRS
NCOL = N_PAIRS + (N_SINGLE - 1) + TAIL_CHUNKS
DET_EPS = 0.001

_NC_CACHE = None


def build_nc(reps=1, tail_chunks=TAIL_CHUNKS, pairs=N_PAIRS):
    """reps>1 repeats the full memory pass for slope benchmarking.

    The first 2*pairs samples are streamed as 2 MiB sample-PAIR transfers: a
    contiguous pair block viewed as [128, 4096] puts sample 2q in partitions
    0..63 and sample 2q+1 in 64..127, so one DMA + one op chain covers two
    samples; the host separates them by partition range. Remaining samples
    stream as 1 MiB singles, with the last sample column-chunked to shrink
    the end-of-stream tail.
    """
    nc = bacc.Bacc()
    f32 = mybir.dt.float32
    n_single = S - 2 * pairs
    ncol = pairs + (n_single - 1) + tail_chunks

    x_ext = nc.declare_dram_parameter("x", [S, P, F], f32, isOutput=False)
    y_ext = nc.declare_dram_parameter("y", [S, P, F], f32, isOutput=False)
    o_ext = nc.declare_dram_parameter("o", [3, P, ncol], f32, isOutput=True)
    # contiguous pair view of the same buffers: [S//2, 128, 2F], where pair q
    # holds sample 2q in partitions 0..63 and sample 2q+1 in 64..127
    x_pair = x_ext.rearrange("(q a) (p2 b) f -> q (a p2) (b f)", a=2, b=2)
    y_pair = y_ext.rearrange("(q a) (p2 b) f -> q (a p2) (b f)", a=2, b=2)

    with tile.TileContext(nc) as tc, ExitStack() as ctx:
        xqp = ctx.enter_context(tc.tile_pool(name="xq", bufs=max(pairs, 1)))
        yqp = ctx.enter_context(tc.tile_pool(name="yq", bufs=max(pairs, 1)))
        xp = ctx.enter_context(tc.tile_pool(name="x", bufs=max(n_single, 1)))
        yp = ctx.enter_context(tc.tile_pool(name="y", bufs=max(n_single, 1)))
        scr = ctx.enter_context(tc.tile_pool(name="scr", bufs=1))
        stat = ctx.enter_context(tc.tile_pool(name="stat", bufs=1))

        sx = stat.tile([P, ncol], f32, tag="sx")
        sy = stat.tile([P, ncol], f32, tag="sy")
        sxy = stat.tile([P, ncol], f32, tag="sxy")

        dve_scr = scr.tile([P, 2 * F], f32, tag="dve_scr")
        act_scr = scr.tile([P, 2 * F], f32, tag="act_scr")

        def chain(col, xt_ap, yt_ap, ds, as_):
            nc.vector.tensor_mul(ds, xt_ap, yt_ap)
            nc.vector.tensor_reduce(
                out=sxy[:, col : col + 1],
                in_=ds,
                axis=mybir.AxisListType.X,
                op=mybir.AluOpType.add,
            )
            nc.scalar.activation(
                out=as_,
                in_=xt_ap,
                func=mybir.ActivationFunctionType.Copy,
                accum_out=sx[:, col : col + 1],
            )
            nc.scalar.activation(
                out=as_,
                in_=yt_ap,
                func=mybir.ActivationFunctionType.Copy,
                accum_out=sy[:, col : col + 1],
            )

        for rep in range(reps):
            for q in range(pairs):
                xt = xqp.tile([P, 2 * F], f32, tag="xq", name=f"xq{rep}_{q}")
                yt = yqp.tile([P, 2 * F], f32, tag="yq", name=f"yq{rep}_{q}")
                nc.sync.dma_start(xt[:], x_pair[q])
                nc.sync.dma_start(yt[:], y_pair[q])
                chain(q, xt[:], yt[:], dve_scr[:], act_scr[:])

            tail_bounds = (
                TAIL_SPLITS
                if tail_chunks == TAIL_CHUNKS
                else tuple(F // tail_chunks * (k + 1) for k in range(tail_chunks))
            )
            for i, s in enumerate(range(2 * pairs, S)):
                bounds = tail_bounds if s == S - 1 else (F,)
                xt = xp.tile([P, F], f32, tag="xt", name=f"xt{rep}_{s}")
                yt = yp.tile([P, F], f32, tag="yt", name=f"yt{rep}_{s}")
                lo = 0
                for c, hi in enumerate(bounds):
                    cs = slice(lo, hi)
                    lo = hi
                    col = pairs + i + c
                    nc.sync.dma_start(xt[:, cs], x_ext[s, :, cs])
                    nc.sync.dma_start(yt[:, cs], y_ext[s, :, cs])
                    chain(col, xt[:, cs], yt[:, cs], dve_scr[:, cs], act_scr[:, cs])

        # Ship per-partition stats; host does the 128-partition sum in fp64.
        nc.sync.dma_start(o_ext[0], sx[:])
        nc.sync.dma_start(o_ext[1], sy[:])
        nc.sync.dma_start(o_ext[2], sxy[:])

    nc.compile()
    return nc


def _get_nc():
    global _NC_CACHE
    if _NC_CACHE is None:
        _NC_CACHE = build_nc()
    return _NC_CACHE


def _device_sums(input, target, trace=False, **kw):
    """Run the Bass kernel; return (sx, sy, sxy) each [B] float64, plus results."""
    x = np.ascontiguousarray(np.asarray(input, dtype=np.float32)).reshape(
        N_CORES, S, P, F
    )
    y = np.ascontiguousarray(np.asarray(target, dtype=np.float32)).reshape(
        N_CORES, S, P, F
    )
    nc = _get_nc()
    in_maps = [{"x": x[c], "y": y[c]} for c in range(N_CORES)]
    res = run_bass_kernel_spmd(nc, in_maps, list(range(N_CORES)), trace=trace, **kw)
    sx = np.empty(B, np.float64)
    sy = np.empty(B, np.float64)
    sxy = np.empty(B, np.float64)

    def unpack(o_t):
        # o_t [P, NCOL] per-partition stats, fp64 partition reduction on host.
        # cols 0..N_PAIRS-1: pair q -> sample 2q in partitions 0:64,
        #   sample 2q+1 in partitions 64:128
        # cols N_PAIRS..: singles, last sample = sum of TAIL_CHUNKS tail cols
        out = np.empty(S, np.float64)
        for q in range(N_PAIRS):
            out[2 * q] = o_t[: P // 2, q].sum()
            out[2 * q + 1] = o_t[P // 2 :, q].sum()
        full = o_t.sum(axis=0)  # [NCOL]
        for i in range(N_SINGLE - 1):
            out[2 * N_PAIRS + i] = full[N_PAIRS + i]
        out[S - 1] = full[N_PAIRS + N_SINGLE - 1 :].sum()
        return out

    for c in range(N_CORES):
        o = np.asarray(res.results[c]["o"], np.float64)  # [3, P, NCOL]
        sx[c * S : (c + 1) * S] = unpack(o[0])
        sy[c * S : (c + 1) * S] = unpack(o[1])
        sxy[c * S : (c + 1) * S] = unpack(o[2])
    return sx, sy, sxy, res


def _loss_from_sums(sx, sy, sxy):
    # mat = [[S_xy, S_x-S_xy], [S_y-S_xy, HW-S_x-S_y+S_xy]]; det = HW*S_xy - S_x*S_y
    m00 = sxy
    m01 = sx - sxy
    m10 = sy - sxy
    m11 = HW - sx - sy + sxy
    det = m00 * m11 - m01 * m10
    loss = -np.log(np.abs(det) + DET_EPS)
    return np.array(loss.mean(), dtype=np.float32)


def kernel(input, target):
    sx, sy, sxy, _ = _device_sums(input, target)
    return _loss_from_sums(sx, sy, sxy)


if __name__ == "__main__":
    rng = np.random.default_rng(0)
    x = rng.random((B, 1, H, W), dtype=np.float32)
    y = rng.random((B, 1, H, W), dtype=np.float32)
    got = kernel(input=x, target=y)
    xf = x.reshape(B, -1).astype(np.float64)
    yf = y.reshape(B, -1).astype(np.float64)
    det = HW * (xf * yf).sum(1) - xf.sum(1) * yf.sum(1)
    want = (-np.log(np.abs(det) + DET_EPS)).mean()
    print("kernel:", got, "numpy:", want, "rel:", abs(got - want) / abs(want))



# revision 26
# speedup vs baseline: 1.0713x; 1.0713x over previous
"""DMI loss kernel for Trainium2 (8 NeuronCores, data-parallel over batch).

reference:
    preds  = [x, 1-x]  [b, 2, hw]
    labels = [y, 1-y]  [b, 2, hw]
    mat    = preds @ labels.T          (per-sample 2x2)
    loss   = mean(-log(|det(mat)| + 1e-3))

Per sample only three reductions over hw are needed:
    S_x = sum(x), S_y = sum(y), S_xy = sum(x*y)
since det(mat) == hw*S_xy - S_x*S_y (exact algebraic identity).

Sharding: batch 64 -> 8 cores x 8 samples. Each core reduces its 8 samples to
partial sums on-device; the det/log/mean epilogue runs on host in float64.

Device pipeline per core (memory-bound; all DMA serializes at ~360 GB/s, so
total time = first-byte latency + 46.6us stream + tail; everything here
attacks the two latencies):
  input : HWDGE transfers on the sync queue. Whole samples for s0-5, sample 6
          in 1024-col blocks, sample 7 in progressively smaller pieces so the
          compute left after the final byte lands is one small piece. Issue
          latency is fully pipelined (~30 issues finish by ~20us while the
          bus runs to ~48us), so small tail transfers cost nothing extra.
  DVE   : ONE fused pass per piece for S_xy (scalar_tensor_tensor computes
          (x*1)*y with free-axis accum_out — tensor_tensor_reduce is not
          executable on this runtime), plus S_x reduces for sample 7's big
          pieces and the final S_y reduce.
  ACT   : activation-copy accumulate for S_x / S_y of samples 0-6 and
          sample 7's early S_y pieces.
  Pool  : XYZWC full-reductions (single-cell results) for sample 7's small
          S_x / S_y pieces — the stats tile is zero-initialized so a
          one-cell total sums identically to a per-partition column on the
          host. (SWDGE prep/trigger DMA would shave another ~1.2us off the
          output chain but dynamic DMA is disabled in this NEFF path: the
          triggered descriptors silently never fire.)
  out   : stats live in one [128, OUTW] tile, shipped by TWO HWDGE DMAs:
          the bulk as soon as the early stats land (overlapped with the
          stream tail), and a 12-column late group so the final
          sem-wait -> HWDGE(625) -> DGE(650) -> transfer -> sem(900) chain
          moves the minimum bytes.
  prelw : the framework's const-AP memsets and entry all-engine barrier are
          excised post-schedule (nothing this kernel references them; Tile
          sem waits carry every real dependency), pulling the first DMA
          issue from ~1.97us to ~1.38us. The second exit barrier (engines
          waiting on Pool's sem clear) is excised the same way.
"""

import sys

for _p in ("/opt/trn_rl_repo",):
    if _p not in sys.path:
        sys.path.append(_p)

import numpy as np
from contextlib import ExitStack

import concourse.bass as bass
import concourse.tile as tile
from concourse import bacc, mybir
from concourse.bass_utils import run_bass_kernel_spmd

N_CORES = 8
B = 64
H = W = 512
HW = H * W
S = B // N_CORES      # samples per core
P = 128               # SBUF partitions
F = HW // P           # free dim per partition
DET_EPS = 0.001

S6_SPLITS = (1024, 2048)
S7_SPLITS = (512, 1024, 1536, 1792, 1920, 2048)
N7 = len(S7_SPLITS)
# engine for each s7 piece's sx / sy sum: 'a'=ACT, 'd'=DVE reduce, 'p'=Pool cell
SX7_ASSIGN = "dddppp"
SY7_ASSIGN = "aaaapd"
LATE_PIECES = 3       # stats of the last LATE_PIECES s7 pieces ship late
OUTW = 48             # stats tile width
LATE = 12             # last LATE columns ship in the late output DMA

_NC_CACHE = None


def _pieces(splits):
    lo, out = 0, []
    for hi in splits:
        out.append((lo, hi))
        lo = hi
    return out


def build_colmap():
    """Column assignment. Returns (colmap, n_early) where colmap is
    kind -> list of (sample, col). Late-piece stats go in [OUTW-LATE, OUTW)."""
    early = iter(range(OUTW - LATE))
    late = iter(range(OUTW - LATE, OUTW))
    cm = {"sx": [], "sy": [], "sxy": []}
    for s in range(6):
        cm["sxy"].append((s, next(early)))
        cm["sx"].append((s, next(early)))
        cm["sy"].append((s, next(early)))
    for _ in _pieces(S6_SPLITS):
        cm["sxy"].append((6, next(early)))
        cm["sx"].append((6, next(early)))
        cm["sy"].append((6, next(early)))
    pieces = _pieces(S7_SPLITS)
    for k in range(len(pieces)):
        group = late if k >= len(pieces) - LATE_PIECES else early
        cm["sxy"].append((7, next(group)))
        cm["sx"].append((7, next(group)))
        cm["sy"].append((7, next(group)))
    return cm


COLMAP = build_colmap()


def build_nc(reps=1, trim_preamble=True, split_out=True, trim_exit=True,
             s7_splits=None, sy7_assign=None, sx7_assign=None, late_pieces=None):
    """reps>1 repeats the full memory pass for slope benchmarking."""
    global S7_SPLITS, SY7_ASSIGN, SX7_ASSIGN, LATE_PIECES, COLMAP
    if s7_splits is not None:
        S7_SPLITS = s7_splits
    if sy7_assign is not None:
        SY7_ASSIGN = sy7_assign
    if sx7_assign is not None:
        SX7_ASSIGN = sx7_assign
    if late_pieces is not None:
        LATE_PIECES = late_pieces
    COLMAP = build_colmap()
    nc = bacc.Bacc()
    f32 = mybir.dt.float32
    ADD = mybir.AluOpType.add
    MULT = mybir.AluOpType.mult

    x_ext = nc.declare_dram_parameter("x", [S, P, F], f32, isOutput=False)
    y_ext = nc.declare_dram_parameter("y", [S, P, F], f32, isOutput=False)
    o_ext = nc.declare_dram_parameter("o", [P, OUTW], f32, isOutput=True)

    with tile.TileContext(nc) as tc, ExitStack() as ctx:
        xp = ctx.enter_context(tc.tile_pool(name="x", bufs=3))
        yp = ctx.enter_context(tc.tile_pool(name="y", bufs=3))
        scr = ctx.enter_context(tc.tile_pool(name="scr", bufs=1))
        stat = ctx.enter_context(tc.tile_pool(name="stat", bufs=1))

        stats = stat.tile([P, OUTW], f32, tag="stats")
        scrd = scr.tile([P, F], f32, tag="scrd")   # DVE product out
        scra = scr.tile([P, F], f32, tag="scra")   # ACT copy out

        nc.gpsimd.memset(stats[:], 0.0)

        def sxy(xt, yt, col, cs=slice(0, F)):
            # fused (x*1)*y with free-axis accumulate: one DVE pass
            nc.vector.scalar_tensor_tensor(
                out=scrd[:, cs], in0=xt[:, cs], scalar=1.0, in1=yt[:, cs],
                op0=MULT, op1=MULT, accum_out=stats[:, col:col + 1])

        def act(t, col, cs=slice(0, F)):
            nc.scalar.activation(
                out=scra[:, cs], in_=t[:, cs],
                func=mybir.ActivationFunctionType.Copy,
                accum_out=stats[:, col:col + 1])

        def pool_cell(t, col, cs):
            # full XYZWC reduction to a single cell in stats row 0; the
            # zero-initialized column sums identically on the host
            nc.gpsimd.tensor_reduce(
                out=stats[0:1, col:col + 1], in_=t[:, cs],
                axis=mybir.AxisListType.XYZWC, op=ADD)

        def dve_red(t, col, cs):
            nc.vector.tensor_reduce(
                out=stats[:, col:col + 1], in_=t[:, cs],
                axis=mybir.AxisListType.X, op=ADD)

        for rep in range(reps):
            cm = build_colmap()
            cxy = iter(c for _, c in cm["sxy"])
            cx = iter(c for _, c in cm["sx"])
            cy = iter(c for _, c in cm["sy"])

            # samples 0-5: whole-sample transfers
            for s in range(6):
                xt = xp.tile([P, F], f32, tag="xt", name=f"xt{rep}_{s}")
                yt = yp.tile([P, F], f32, tag="yt", name=f"yt{rep}_{s}")
                nc.sync.dma_start(xt[:], x_ext[s])
                nc.sync.dma_start(yt[:], y_ext[s])
                sxy(xt, yt, next(cxy))
                act(xt, next(cx))
                act(yt, next(cy))

            # sample 6: 1024-col blocks
            x6 = xp.tile([P, F], f32, tag="xt", name=f"xt{rep}_6")
            y6 = yp.tile([P, F], f32, tag="yt", name=f"yt{rep}_6")
            lo = 0
            for hi in S6_SPLITS:
                cs = slice(lo, hi)
                lo = hi
                nc.sync.dma_start(x6[:, cs], x_ext[6, :, cs])
                nc.sync.dma_start(y6[:, cs], y_ext[6, :, cs])
                sxy(x6, y6, next(cxy), cs)
                act(x6, next(cx), cs)
                act(y6, next(cy), cs)

            # sample 7: shrinking pieces; sx on Pool cells, sy per SY7_ASSIGN
            x7 = xp.tile([P, F], f32, tag="xt", name=f"xt{rep}_7")
            y7 = yp.tile([P, F], f32, tag="yt", name=f"yt{rep}_7")
            def do_sum(a, t, col, cs):
                if a == "a":
                    act(t, col, cs)
                elif a == "d":
                    dve_red(t, col, cs)
                else:
                    pool_cell(t, col, cs)

            for k, (lo, hi) in enumerate(_pieces(S7_SPLITS)):
                cs = slice(lo, hi)
                nc.sync.dma_start(x7[:, cs], x_ext[7, :, cs])
                nc.sync.dma_start(y7[:, cs], y_ext[7, :, cs])
                do_sum(SX7_ASSIGN[k], x7, next(cx), cs)
                sxy(x7, y7, next(cxy), cs)
                do_sum(SY7_ASSIGN[k], y7, next(cy), cs)

            if rep == reps - 1:
                if split_out:
                    nc.sync.dma_start(o_ext[:, : OUTW - LATE],
                                      stats[:, : OUTW - LATE])
                    nc.sync.dma_start(o_ext[:, OUTW - LATE:],
                                      stats[:, OUTW - LATE:])
                else:
                    nc.sync.dma_start(o_ext[:, :], stats[:])

    if trim_preamble:
        # The four const-AP memsets are unreferenced, and the entry
        # all-engine barrier only orders them before the body; every real
        # dependency is a Tile sem wait (sems were cleared by the previous
        # run's exit). The barrier protocol is inc-then-sub within each
        # instance, so removing the whole entry instance leaves the exit
        # barriers' thresholds intact.
        entry = nc.m.functions[0].blocks[0]
        keep = []
        for ins in entry.instructions:
            tn = type(ins).__name__
            if tn == "InstMemset":
                continue
            if tn == "InstEventSemaphore" and ins.name.startswith("barrier_"):
                continue
            if tn == "InstDrain" and ins.sync_info is not None:
                ins.sync_info.on_wait = []
                ins.sync_info.on_update = []
            keep.append(ins)
        entry.instructions = keep

    if trim_exit:
        # Drop the second exit all-engine barrier (after the sem clear): it
        # only makes the non-Pool engines wait for Pool's semaphore clear,
        # which nothing downstream of them needs — each queue simply ends.
        # The barrier instance is inc-then-sub balanced, so removing all of
        # its instructions keeps the sem protocol consistent.
        import re as _re
        found = []  # (number, name)
        for fn in nc.m.functions:
            for bb in fn.blocks:
                for ins in bb.instructions:
                    if (type(ins).__name__ == "InstEventSemaphore"
                            and ins.name.startswith("barrier_")):
                        m = _re.match(r"barrier_[A-Za-z]+_(\d+)", ins.name)
                        if m:
                            found.append((int(m.group(1)), ins.name))
        found.sort()
        # one barrier instance = 6 instructions (4 engine halves + Pool pair)
        if len(found) >= 6:
            doomed = {n for _, n in found[-6:]}
            for fn in nc.m.functions:
                for bb in fn.blocks:
                    bb.instructions = [i for i in bb.instructions
                                       if i.name not in doomed]

    nc.compile()
    return nc


def _get_nc():
    global _NC_CACHE
    if _NC_CACHE is None:
        _NC_CACHE = build_nc()
    return _NC_CACHE


def _device_sums(input, target, trace=False, **kw):
    """Run the Bass kernel; return (sx, sy, sxy) each [B] float64, plus results."""
    x = np.ascontiguousarray(np.asarray(input, dtype=np.float32)).reshape(
        N_CORES, S, P, F
    )
    y = np.ascontiguousarray(np.asarray(target, dtype=np.float32)).reshape(
        N_CORES, S, P, F
    )
    nc = _get_nc()
    in_maps = [{"x": x[c], "y": y[c]} for c in range(N_CORES)]
    res = run_bass_kernel_spmd(nc, in_maps, list(range(N_CORES)), trace=trace, **kw)
    sx = np.zeros(B, np.float64)
    sy = np.zeros(B, np.float64)
    sxy = np.zeros(B, np.float64)
    for c in range(N_CORES):
        o = np.asarray(res.results[c]["o"], np.float64)  # [P, OUTW]
        cols = o.sum(axis=0)
        for kind, arr in (("sx", sx), ("sy", sy), ("sxy", sxy)):
            for s, col in COLMAP[kind]:
                arr[c * S + s] += cols[col]
    return sx, sy, sxy, res


def _loss_from_sums(sx, sy, sxy):
    # mat = [[S_xy, S_x-S_xy], [S_y-S_xy, HW-S_x-S_y+S_xy]]; det = HW*S_xy - S_x*S_y
    det = HW * sxy - sx * sy
    loss = -np.log(np.abs(det) + DET_EPS)
    return np.array(loss.mean(), dtype=np.float32)


def kernel(input, target):
    sx, sy, sxy, _ = _device_sums(input, target)
    return _loss_from_sums(sx, sy, sxy)


if __name__ == "__main__":
    rng = np.random.default_rng(0)
    x = rng.random((B, 1, H, W), dtype=np.float32)
    y = rng.random((B, 1, H, W), dtype=np.float32)
    got = kernel(input=x, target=y)
    xf = x.reshape(B, -1).astype(np.float64)
    yf = y.reshape(B, -1).astype(np.float64)
    det = HW * (xf * yf).sum(1) - xf.sum(1) * yf.sum(1)
    want = (-np.log(np.abs(det) + DET_EPS)).mean()
    print("kernel:", got, "numpy:", want, "rel:", abs(got - want) / abs(want))


# revision 31
# speedup vs baseline: 1.0748x; 1.0033x over previous
"""DMI loss kernel for Trainium2 (8 NeuronCores, data-parallel over batch).

reference:
    preds  = [x, 1-x]  [b, 2, hw]
    labels = [y, 1-y]  [b, 2, hw]
    mat    = preds @ labels.T          (per-sample 2x2)
    loss   = mean(-log(|det(mat)| + 1e-3))

Per sample only three reductions over hw are needed:
    S_x = sum(x), S_y = sum(y), S_xy = sum(x*y)
since det(mat) == hw*S_xy - S_x*S_y (exact algebraic identity).

Sharding: batch 64 -> 8 cores x 8 samples. Each core reduces its 8 samples to
partial sums on-device; the det/log/mean epilogue runs on host in float64.

Device pipeline per core (memory-bound; all DMA serializes at ~360 GB/s, so
total time = first-byte latency + 46.6us stream + tail; everything here
attacks the two latencies):
  input : HWDGE transfers on the sync queue. Whole samples for s0-5, sample 6
          in 1024-col blocks, sample 7 in progressively smaller pieces so the
          compute left after the final byte lands is one small piece. Issue
          latency is fully pipelined (~30 issues finish by ~20us while the
          bus runs to ~48us), so small tail transfers cost nothing extra.
  DVE   : ONE fused pass per piece for S_xy (scalar_tensor_tensor computes
          (x*1)*y with free-axis accum_out — tensor_tensor_reduce is not
          executable on this runtime), plus S_x reduces for sample 7's big
          pieces and the final S_y reduce.
  ACT   : activation-copy accumulate for S_x / S_y of samples 0-6 and
          sample 7's early S_y pieces.
  Pool  : XYZWC full-reductions (single-cell results) for sample 7's small
          S_x / S_y pieces — the stats tile is zero-initialized so a
          one-cell total sums identically to a per-partition column on the
          host. (SWDGE prep/trigger DMA would shave another ~1.2us off the
          output chain but dynamic DMA is disabled in this NEFF path: the
          triggered descriptors silently never fire.)
  out   : stats live in one [128, OUTW] tile, shipped by TWO HWDGE DMAs:
          the bulk as soon as the early stats land (overlapped with the
          stream tail), and a 12-column late group so the final
          sem-wait -> HWDGE(625) -> DGE(650) -> transfer -> sem(900) chain
          moves the minimum bytes.
  prelw : the framework's const-AP memsets and entry all-engine barrier are
          excised post-schedule (nothing this kernel references them; Tile
          sem waits carry every real dependency), pulling the first DMA
          issue from ~1.97us to ~1.38us. The second exit barrier (engines
          waiting on Pool's sem clear) is excised the same way.
"""

import sys

for _p in ("/opt/trn_rl_repo",):
    if _p not in sys.path:
        sys.path.append(_p)

import numpy as np
from contextlib import ExitStack

import concourse.bass as bass
import concourse.tile as tile
from concourse import bacc, mybir
from concourse.bass_utils import run_bass_kernel_spmd

N_CORES = 8
B = 64
H = W = 512
HW = H * W
S = B // N_CORES      # samples per core
P = 128               # SBUF partitions
F = HW // P           # free dim per partition
DET_EPS = 0.001

S6_SPLITS = (1024, 2048)
S7_SPLITS = (512, 1024, 1536, 1792, 1920, 2048)
N7 = len(S7_SPLITS)
# engine for each s7 piece's sx / sy sum: 'a'=ACT, 'd'=DVE reduce, 'p'=Pool cell
SX7_ASSIGN = "dddppd"
SY7_ASSIGN = "aaaapp"
LATE_PIECES = 3       # stats of the last LATE_PIECES s7 pieces ship late
OUTW = 48             # total output width
LATE = 12             # last LATE columns live in the late tile / late DMA

_NC_CACHE = None


def _pieces(splits):
    lo, out = 0, []
    for hi in splits:
        out.append((lo, hi))
        lo = hi
    return out


def build_colmap():
    """Column assignment. Returns (colmap, n_early) where colmap is
    kind -> list of (sample, col). Late-piece stats go in [OUTW-LATE, OUTW)."""
    early = iter(range(OUTW - LATE))
    late = iter(range(OUTW - LATE, OUTW))
    cm = {"sx": [], "sy": [], "sxy": []}
    for s in range(6):
        cm["sxy"].append((s, next(early)))
        cm["sx"].append((s, next(early)))
        cm["sy"].append((s, next(early)))
    for _ in _pieces(S6_SPLITS):
        cm["sxy"].append((6, next(early)))
        cm["sx"].append((6, next(early)))
        cm["sy"].append((6, next(early)))
    pieces = _pieces(S7_SPLITS)
    for k in range(len(pieces)):
        group = late if k >= len(pieces) - LATE_PIECES else early
        cm["sxy"].append((7, next(group)))
        cm["sx"].append((7, next(group)))
        cm["sy"].append((7, next(group)))
    return cm


COLMAP = build_colmap()


def build_nc(reps=1, trim_preamble=True, split_out=True, trim_exit=True,
             s7_splits=None, sy7_assign=None, sx7_assign=None, late_pieces=None):
    """reps>1 repeats the full memory pass for slope benchmarking."""
    global S7_SPLITS, SY7_ASSIGN, SX7_ASSIGN, LATE_PIECES, COLMAP
    if s7_splits is not None:
        S7_SPLITS = s7_splits
    if sy7_assign is not None:
        SY7_ASSIGN = sy7_assign
    if sx7_assign is not None:
        SX7_ASSIGN = sx7_assign
    if late_pieces is not None:
        LATE_PIECES = late_pieces
    COLMAP = build_colmap()
    nc = bacc.Bacc()
    f32 = mybir.dt.float32
    ADD = mybir.AluOpType.add
    MULT = mybir.AluOpType.mult

    x_ext = nc.declare_dram_parameter("x", [S, P, F], f32, isOutput=False)
    y_ext = nc.declare_dram_parameter("y", [S, P, F], f32, isOutput=False)
    o_ext = nc.declare_dram_parameter("o", [P, OUTW], f32, isOutput=True)

    with tile.TileContext(nc) as tc, ExitStack() as ctx:
        xp = ctx.enter_context(tc.tile_pool(name="x", bufs=3))
        yp = ctx.enter_context(tc.tile_pool(name="y", bufs=3))
        scr = ctx.enter_context(tc.tile_pool(name="scr", bufs=1))
        stat = ctx.enter_context(tc.tile_pool(name="stat", bufs=1))

        # Two PHYSICAL stats tiles: Tile tracks a DMA's tile read coarsely, so
        # a single tile would gate the early output DMA on the late writers.
        stats_e = stat.tile([P, OUTW - LATE], f32, tag="stats_e")
        stats_l = stat.tile([P, LATE], f32, tag="stats_l")
        scrd = scr.tile([P, F], f32, tag="scrd")   # DVE product out
        scra = scr.tile([P, F], f32, tag="scra")   # ACT copy out

        def col_ap(col):
            if col < OUTW - LATE:
                return stats_e[:, col:col + 1]
            return stats_l[:, col - (OUTW - LATE):col - (OUTW - LATE) + 1]

        def cell_ap(col):
            if col < OUTW - LATE:
                return stats_e[0:1, col:col + 1]
            return stats_l[0:1, col - (OUTW - LATE):col - (OUTW - LATE) + 1]

        nc.gpsimd.memset(stats_e[:], 0.0)
        nc.gpsimd.memset(stats_l[:], 0.0)

        def sxy(xt, yt, col, cs=slice(0, F)):
            # fused (x*1)*y with free-axis accumulate: one DVE pass
            nc.vector.scalar_tensor_tensor(
                out=scrd[:, cs], in0=xt[:, cs], scalar=1.0, in1=yt[:, cs],
                op0=MULT, op1=MULT, accum_out=col_ap(col))

        def act(t, col, cs=slice(0, F)):
            nc.scalar.activation(
                out=scra[:, cs], in_=t[:, cs],
                func=mybir.ActivationFunctionType.Copy,
                accum_out=col_ap(col))

        def pool_cell(t, col, cs):
            # full XYZWC reduction to a single cell in stats row 0; the
            # zero-initialized column sums identically on the host
            nc.gpsimd.tensor_reduce(
                out=cell_ap(col), in_=t[:, cs],
                axis=mybir.AxisListType.XYZWC, op=ADD)

        def dve_red(t, col, cs):
            nc.vector.tensor_reduce(
                out=col_ap(col), in_=t[:, cs],
                axis=mybir.AxisListType.X, op=ADD)

        for rep in range(reps):
            cm = build_colmap()
            cxy = iter(c for _, c in cm["sxy"])
            cx = iter(c for _, c in cm["sx"])
            cy = iter(c for _, c in cm["sy"])

            # samples 0-5: whole-sample transfers
            for s in range(6):
                xt = xp.tile([P, F], f32, tag="xt", name=f"xt{rep}_{s}")
                yt = yp.tile([P, F], f32, tag="yt", name=f"yt{rep}_{s}")
                nc.sync.dma_start(xt[:], x_ext[s])
                nc.sync.dma_start(yt[:], y_ext[s])
                sxy(xt, yt, next(cxy))
                act(xt, next(cx))
                act(yt, next(cy))

            # sample 6: 1024-col blocks
            x6 = xp.tile([P, F], f32, tag="xt", name=f"xt{rep}_6")
            y6 = yp.tile([P, F], f32, tag="yt", name=f"yt{rep}_6")
            lo = 0
            for hi in S6_SPLITS:
                cs = slice(lo, hi)
                lo = hi
                nc.sync.dma_start(x6[:, cs], x_ext[6, :, cs])
                nc.sync.dma_start(y6[:, cs], y_ext[6, :, cs])
                sxy(x6, y6, next(cxy), cs)
                act(x6, next(cx), cs)
                act(y6, next(cy), cs)

            # sample 7: shrinking pieces; sx on Pool cells, sy per SY7_ASSIGN
            x7 = xp.tile([P, F], f32, tag="xt", name=f"xt{rep}_7")
            y7 = yp.tile([P, F], f32, tag="yt", name=f"yt{rep}_7")
            def do_sum(a, t, col, cs):
                if a == "a":
                    act(t, col, cs)
                elif a == "d":
                    dve_red(t, col, cs)
                else:
                    pool_cell(t, col, cs)

            for k, (lo, hi) in enumerate(_pieces(S7_SPLITS)):
                cs = slice(lo, hi)
                nc.sync.dma_start(x7[:, cs], x_ext[7, :, cs])
                nc.sync.dma_start(y7[:, cs], y_ext[7, :, cs])
                do_sum(SX7_ASSIGN[k], x7, next(cx), cs)
                sxy(x7, y7, next(cxy), cs)
                do_sum(SY7_ASSIGN[k], y7, next(cy), cs)
                if rep == reps - 1 and k == N7 - LATE_PIECES - 1:
                    # ship the early stats NOW, in program order before the
                    # late-piece ops, so the in-order SP queue can't trap this
                    # DMA behind a wait on the late writers
                    nc.sync.dma_start(o_ext[:, : OUTW - LATE], stats_e[:])

            if rep == reps - 1:
                nc.sync.dma_start(o_ext[:, OUTW - LATE:], stats_l[:])

    if trim_preamble:
        # The four const-AP memsets are unreferenced, and the entry
        # all-engine barrier only orders them before the body; every real
        # dependency is a Tile sem wait (sems were cleared by the previous
        # run's exit). The barrier protocol is inc-then-sub within each
        # instance, so removing the whole entry instance leaves the exit
        # barriers' thresholds intact.
        entry = nc.m.functions[0].blocks[0]
        keep = []
        for ins in entry.instructions:
            tn = type(ins).__name__
            if tn == "InstMemset":
                continue
            if tn == "InstEventSemaphore" and ins.name.startswith("barrier_"):
                continue
            if tn == "InstDrain" and ins.sync_info is not None:
                ins.sync_info.on_wait = []
                ins.sync_info.on_update = []
            keep.append(ins)
        entry.instructions = keep

    if trim_exit:
        # Drop the second exit all-engine barrier (after the sem clear): it
        # only makes the non-Pool engines wait for Pool's semaphore clear,
        # which nothing downstream of them needs — each queue simply ends.
        # The barrier instance is inc-then-sub balanced, so removing all of
        # its instructions keeps the sem protocol consistent.
        import re as _re
        found = []  # (number, name)
        for fn in nc.m.functions:
            for bb in fn.blocks:
                for ins in bb.instructions:
                    if (type(ins).__name__ == "InstEventSemaphore"
                            and ins.name.startswith("barrier_")):
                        m = _re.match(r"barrier_[A-Za-z]+_(\d+)", ins.name)
                        if m:
                            found.append((int(m.group(1)), ins.name))
        found.sort()
        # one barrier instance = 6 instructions (4 engine halves + Pool pair)
        if len(found) >= 6:
            doomed = {n for _, n in found[-6:]}
            for fn in nc.m.functions:
                for bb in fn.blocks:
                    bb.instructions = [i for i in bb.instructions
                                       if i.name not in doomed]

    nc.compile()
    return nc


def _get_nc():
    global _NC_CACHE
    if _NC_CACHE is None:
        _NC_CACHE = build_nc()
    return _NC_CACHE


def _device_sums(input, target, trace=False, **kw):
    """Run the Bass kernel; return (sx, sy, sxy) each [B] float64, plus results."""
    x = np.ascontiguousarray(np.asarray(input, dtype=np.float32)).reshape(
        N_CORES, S, P, F
    )
    y = np.ascontiguousarray(np.asarray(target, dtype=np.float32)).reshape(
        N_CORES, S, P, F
    )
    nc = _get_nc()
    in_maps = [{"x": x[c], "y": y[c]} for c in range(N_CORES)]
    res = run_bass_kernel_spmd(nc, in_maps, list(range(N_CORES)), trace=trace, **kw)
    sx = np.zeros(B, np.float64)
    sy = np.zeros(B, np.float64)
    sxy = np.zeros(B, np.float64)
    for c in range(N_CORES):
        o = np.asarray(res.results[c]["o"], np.float64)  # [P, OUTW]
        cols = o.sum(axis=0)
        for kind, arr in (("sx", sx), ("sy", sy), ("sxy", sxy)):
            for s, col in COLMAP[kind]:
                arr[c * S + s] += cols[col]
    return sx, sy, sxy, res


def _loss_from_sums(sx, sy, sxy):
    # mat = [[S_xy, S_x-S_xy], [S_y-S_xy, HW-S_x-S_y+S_xy]]; det = HW*S_xy - S_x*S_y
    det = HW * sxy - sx * sy
    loss = -np.log(np.abs(det) + DET_EPS)
    return np.array(loss.mean(), dtype=np.float32)


def kernel(input, target):
    sx, sy, sxy, _ = _device_sums(input, target)
    return _loss_from_sums(sx, sy, sxy)


if __name__ == "__main__":
    rng = np.random.default_rng(0)
    x = rng.random((B, 1, H, W), dtype=np.float32)
    y = rng.random((B, 1, H, W), dtype=np.float32)
    got = kernel(input=x, target=y)
    xf = x.reshape(B, -1).astype(np.float64)
    yf = y.reshape(B, -1).astype(np.float64)
    det = HW * (xf * yf).sum(1) - xf.sum(1) * yf.sum(1)
    want = (-np.log(np.abs(det) + DET_EPS)).mean()
    print("kernel:", got, "numpy:", want, "rel:", abs(got - want) / abs(want))


# revision 32
# speedup vs baseline: 1.0754x; 1.0006x over previous
"""DMI loss kernel for Trainium2 (8 NeuronCores, data-parallel over batch).

reference:
    preds  = [x, 1-x]  [b, 2, hw]
    labels = [y, 1-y]  [b, 2, hw]
    mat    = preds @ labels.T          (per-sample 2x2)
    loss   = mean(-log(|det(mat)| + 1e-3))

Per sample only three reductions over hw are needed:
    S_x = sum(x), S_y = sum(y), S_xy = sum(x*y)
since det(mat) == hw*S_xy - S_x*S_y (exact algebraic identity).

Sharding: batch 64 -> 8 cores x 8 samples. Each core reduces its 8 samples to
partial sums on-device; the det/log/mean epilogue runs on host in float64.

Device pipeline per core (memory-bound; all DMA serializes at ~360 GB/s, so
total time = first-byte latency + 46.6us stream + tail; everything here
attacks the two latencies):
  input : HWDGE transfers on the sync queue. Whole samples for s0-5, sample 6
          in 1024-col blocks, sample 7 in progressively smaller pieces so the
          compute left after the final byte lands is one small piece. Issue
          latency is fully pipelined (~30 issues finish by ~20us while the
          bus runs to ~48us), so small tail transfers cost nothing extra.
  DVE   : ONE fused pass per piece for S_xy (scalar_tensor_tensor computes
          (x*1)*y with free-axis accum_out — tensor_tensor_reduce is not
          executable on this runtime), plus S_x reduces for sample 7's big
          pieces and the final S_y reduce.
  ACT   : activation-copy accumulate for S_x / S_y of samples 0-6 and
          sample 7's early S_y pieces.
  Pool  : XYZWC full-reductions (single-cell results) for sample 7's small
          S_x / S_y pieces — the stats tile is zero-initialized so a
          one-cell total sums identically to a per-partition column on the
          host. (SWDGE prep/trigger DMA would shave another ~1.2us off the
          output chain but dynamic DMA is disabled in this NEFF path: the
          triggered descriptors silently never fire.)
  out   : stats live in one [128, OUTW] tile, shipped by TWO HWDGE DMAs:
          the bulk as soon as the early stats land (overlapped with the
          stream tail), and a 12-column late group so the final
          sem-wait -> HWDGE(625) -> DGE(650) -> transfer -> sem(900) chain
          moves the minimum bytes.
  prelw : the framework's const-AP memsets and entry all-engine barrier are
          excised post-schedule (nothing this kernel references them; Tile
          sem waits carry every real dependency), pulling the first DMA
          issue from ~1.97us to ~1.38us. The second exit barrier (engines
          waiting on Pool's sem clear) is excised the same way.
"""

import sys

for _p in ("/opt/trn_rl_repo",):
    if _p not in sys.path:
        sys.path.append(_p)

import numpy as np
from contextlib import ExitStack

import concourse.bass as bass
import concourse.tile as tile
from concourse import bacc, mybir
from concourse.bass_utils import run_bass_kernel_spmd

N_CORES = 8
B = 64
H = W = 512
HW = H * W
S = B // N_CORES      # samples per core
P = 128               # SBUF partitions
F = HW // P           # free dim per partition
DET_EPS = 0.001

S6_SPLITS = (1024, 2048)
S7_SPLITS = (512, 1024, 1536, 1792, 1920, 2048)
N7 = len(S7_SPLITS)
# engine for each s7 piece's sx / sy sum: 'a'=ACT, 'd'=DVE reduce, 'p'=Pool cell
SX7_ASSIGN = "dddppd"
SY7_ASSIGN = "aaaapp"
LATE_PIECES = 3       # stats of the last LATE_PIECES s7 pieces ship late
OUTW = 48             # total output width
LATE = 12             # last LATE columns live in the late tile / late DMA

_NC_CACHE = None


def _pieces(splits):
    lo, out = 0, []
    for hi in splits:
        out.append((lo, hi))
        lo = hi
    return out


def build_colmap():
    """Column assignment. Returns (colmap, n_early) where colmap is
    kind -> list of (sample, col). Late-piece stats go in [OUTW-LATE, OUTW)."""
    early = iter(range(OUTW - LATE))
    late = iter(range(OUTW - LATE, OUTW))
    cm = {"sx": [], "sy": [], "sxy": []}
    for s in range(6):
        cm["sxy"].append((s, next(early)))
        cm["sx"].append((s, next(early)))
        cm["sy"].append((s, next(early)))
    for _ in _pieces(S6_SPLITS):
        cm["sxy"].append((6, next(early)))
        cm["sx"].append((6, next(early)))
        cm["sy"].append((6, next(early)))
    pieces = _pieces(S7_SPLITS)
    for k in range(len(pieces)):
        group = late if k >= len(pieces) - LATE_PIECES else early
        cm["sxy"].append((7, next(group)))
        cm["sx"].append((7, next(group)))
        cm["sy"].append((7, next(group)))
    return cm


COLMAP = build_colmap()


def build_nc(reps=1, trim_preamble=True, split_out=True, trim_exit=True,
             s7_splits=None, sy7_assign=None, sx7_assign=None, late_pieces=None):
    """reps>1 repeats the full memory pass for slope benchmarking."""
    global S7_SPLITS, SY7_ASSIGN, SX7_ASSIGN, LATE_PIECES, COLMAP
    if s7_splits is not None:
        S7_SPLITS = s7_splits
    if sy7_assign is not None:
        SY7_ASSIGN = sy7_assign
    if sx7_assign is not None:
        SX7_ASSIGN = sx7_assign
    if late_pieces is not None:
        LATE_PIECES = late_pieces
    COLMAP = build_colmap()
    nc = bacc.Bacc()
    f32 = mybir.dt.float32
    ADD = mybir.AluOpType.add
    MULT = mybir.AluOpType.mult

    x_ext = nc.declare_dram_parameter("x", [S, P, F], f32, isOutput=False)
    y_ext = nc.declare_dram_parameter("y", [S, P, F], f32, isOutput=False)
    o_ext = nc.declare_dram_parameter("o", [P, OUTW], f32, isOutput=True)

    with tile.TileContext(nc) as tc, ExitStack() as ctx:
        xp = ctx.enter_context(tc.tile_pool(name="x", bufs=3))
        yp = ctx.enter_context(tc.tile_pool(name="y", bufs=3))
        scr = ctx.enter_context(tc.tile_pool(name="scr", bufs=1))
        stat = ctx.enter_context(tc.tile_pool(name="stat", bufs=1))

        # Two PHYSICAL stats tiles: Tile tracks a DMA's tile read coarsely, so
        # a single tile would gate the early output DMA on the late writers.
        stats_e = stat.tile([P, OUTW - LATE], f32, tag="stats_e")
        stats_l = stat.tile([P, LATE], f32, tag="stats_l")
        scrd = scr.tile([P, F], f32, tag="scrd")   # DVE product out
        scra = scr.tile([P, F], f32, tag="scra")   # ACT copy out

        def col_ap(col):
            if col < OUTW - LATE:
                return stats_e[:, col:col + 1]
            return stats_l[:, col - (OUTW - LATE):col - (OUTW - LATE) + 1]

        def cell_ap(col):
            if col < OUTW - LATE:
                return stats_e[0:1, col:col + 1]
            return stats_l[0:1, col - (OUTW - LATE):col - (OUTW - LATE) + 1]

        nc.gpsimd.memset(stats_e[:], 0.0)
        nc.gpsimd.memset(stats_l[:], 0.0)

        def sxy(xt, yt, col, cs=slice(0, F)):
            # fused (x*1)*y with free-axis accumulate: one DVE pass
            nc.vector.scalar_tensor_tensor(
                out=scrd[:, cs], in0=xt[:, cs], scalar=1.0, in1=yt[:, cs],
                op0=MULT, op1=MULT, accum_out=col_ap(col))

        def act(t, col, cs=slice(0, F)):
            nc.scalar.activation(
                out=scra[:, cs], in_=t[:, cs],
                func=mybir.ActivationFunctionType.Copy,
                accum_out=col_ap(col))

        def pool_cell(t, col, cs):
            # full XYZWC reduction to a single cell in stats row 0; the
            # zero-initialized column sums identically on the host
            nc.gpsimd.tensor_reduce(
                out=cell_ap(col), in_=t[:, cs],
                axis=mybir.AxisListType.XYZWC, op=ADD)

        def dve_red(t, col, cs):
            nc.vector.tensor_reduce(
                out=col_ap(col), in_=t[:, cs],
                axis=mybir.AxisListType.X, op=ADD)

        for rep in range(reps):
            cm = build_colmap()
            cxy = iter(c for _, c in cm["sxy"])
            cx = iter(c for _, c in cm["sx"])
            cy = iter(c for _, c in cm["sy"])

            # samples 0-5: whole-sample transfers
            for s in range(6):
                xt = xp.tile([P, F], f32, tag="xt", name=f"xt{rep}_{s}")
                yt = yp.tile([P, F], f32, tag="yt", name=f"yt{rep}_{s}")
                nc.sync.dma_start(xt[:], x_ext[s])
                nc.sync.dma_start(yt[:], y_ext[s])
                sxy(xt, yt, next(cxy))
                act(xt, next(cx))
                act(yt, next(cy))

            # sample 6: 1024-col blocks
            x6 = xp.tile([P, F], f32, tag="xt", name=f"xt{rep}_6")
            y6 = yp.tile([P, F], f32, tag="yt", name=f"yt{rep}_6")
            lo = 0
            for hi in S6_SPLITS:
                cs = slice(lo, hi)
                lo = hi
                nc.sync.dma_start(x6[:, cs], x_ext[6, :, cs])
                nc.sync.dma_start(y6[:, cs], y_ext[6, :, cs])
                sxy(x6, y6, next(cxy), cs)
                act(x6, next(cx), cs)
                act(y6, next(cy), cs)

            # sample 7: shrinking pieces; sx on Pool cells, sy per SY7_ASSIGN
            x7 = xp.tile([P, F], f32, tag="xt", name=f"xt{rep}_7")
            y7 = yp.tile([P, F], f32, tag="yt", name=f"yt{rep}_7")
            def do_sum(a, t, col, cs):
                if a == "a":
                    act(t, col, cs)
                elif a == "d":
                    dve_red(t, col, cs)
                else:
                    pool_cell(t, col, cs)

            for k, (lo, hi) in enumerate(_pieces(S7_SPLITS)):
                cs = slice(lo, hi)
                nc.sync.dma_start(x7[:, cs], x_ext[7, :, cs])
                nc.sync.dma_start(y7[:, cs], y_ext[7, :, cs])
                do_sum(SX7_ASSIGN[k], x7, next(cx), cs)
                sxy(x7, y7, next(cxy), cs)
                do_sum(SY7_ASSIGN[k], y7, next(cy), cs)
                if rep == reps - 1 and k == N7 - LATE_PIECES - 1:
                    # ship the early stats NOW, in program order before the
                    # late-piece ops, so the in-order SP queue can't trap this
                    # DMA behind a wait on the late writers
                    nc.sync.dma_start(o_ext[:, : OUTW - LATE], stats_e[:])

            if rep == reps - 1:
                nc.sync.dma_start(o_ext[:, OUTW - LATE:], stats_l[:])

    if trim_preamble:
        # The four const-AP memsets are unreferenced, and the entry
        # all-engine barrier only orders them before the body; every real
        # dependency is a Tile sem wait (sems were cleared by the previous
        # run's exit). The barrier protocol is inc-then-sub within each
        # instance, so removing the whole entry instance leaves the exit
        # barriers' thresholds intact.
        entry = nc.m.functions[0].blocks[0]
        keep = []
        for ins in entry.instructions:
            tn = type(ins).__name__
            if tn == "InstMemset":
                continue
            if tn == "InstEventSemaphore" and ins.name.startswith("barrier_"):
                continue
            if tn == "InstDrain" and ins.sync_info is not None:
                ins.sync_info.on_wait = []
                ins.sync_info.on_update = []
            keep.append(ins)
        entry.instructions = keep

    if trim_exit:
        # Drop the second exit all-engine barrier (after the sem clear): it
        # only makes the non-Pool engines wait for Pool's semaphore clear,
        # which nothing downstream of them needs — each queue simply ends.
        # The barrier instance is inc-then-sub balanced, so removing all of
        # its instructions keeps the sem protocol consistent.
        import re as _re
        found = []  # (number, name)
        for fn in nc.m.functions:
            for bb in fn.blocks:
                for ins in bb.instructions:
                    if (type(ins).__name__ == "InstEventSemaphore"
                            and ins.name.startswith("barrier_")):
                        m = _re.match(r"barrier_[A-Za-z]+_(\d+)", ins.name)
                        if m:
                            found.append((int(m.group(1)), ins.name))
        found.sort()
        # one barrier instance = 6 instructions (4 engine halves + Pool pair)
        if len(found) >= 6:
            doomed = {n for _, n in found[-6:]}
            for fn in nc.m.functions:
                for bb in fn.blocks:
                    bb.instructions = [i for i in bb.instructions
                                       if i.name not in doomed]
            # The removed barrier's Drain halves carry gather+1 updates and
            # release==0 waits; with the EventSemaphore halves gone those
            # updates would leak +4 onto the persistent barrier sem every
            # run. Strip the sync_info of every Drain AFTER the sem-clear
            # ISA so the removal is balanced.
            for fn in nc.m.functions:
                for bb in fn.blocks:
                    seen_clear = False
                    for ins in bb.instructions:
                        tn = type(ins).__name__
                        if tn == "InstISA":
                            seen_clear = True
                        elif (seen_clear and tn == "InstDrain"
                                and ins.sync_info is not None):
                            ins.sync_info.on_wait = []
                            ins.sync_info.on_update = []

    nc.compile()
    return nc


def _get_nc():
    global _NC_CACHE
    if _NC_CACHE is None:
        _NC_CACHE = build_nc()
    return _NC_CACHE


def _device_sums(input, target, trace=False, **kw):
    """Run the Bass kernel; return (sx, sy, sxy) each [B] float64, plus results."""
    x = np.ascontiguousarray(np.asarray(input, dtype=np.float32)).reshape(
        N_CORES, S, P, F
    )
    y = np.ascontiguousarray(np.asarray(target, dtype=np.float32)).reshape(
        N_CORES, S, P, F
    )
    nc = _get_nc()
    in_maps = [{"x": x[c], "y": y[c]} for c in range(N_CORES)]
    res = run_bass_kernel_spmd(nc, in_maps, list(range(N_CORES)), trace=trace, **kw)
    sx = np.zeros(B, np.float64)
    sy = np.zeros(B, np.float64)
    sxy = np.zeros(B, np.float64)
    for c in range(N_CORES):
        o = np.asarray(res.results[c]["o"], np.float64)  # [P, OUTW]
        cols = o.sum(axis=0)
        for kind, arr in (("sx", sx), ("sy", sy), ("sxy", sxy)):
            for s, col in COLMAP[kind]:
                arr[c * S + s] += cols[col]
    return sx, sy, sxy, res


def _loss_from_sums(sx, sy, sxy):
    # mat = [[S_xy, S_x-S_xy], [S_y-S_xy, HW-S_x-S_y+S_xy]]; det = HW*S_xy - S_x*S_y
    det = HW * sxy - sx * sy
    loss = -np.log(np.abs(det) + DET_EPS)
    return np.array(loss.mean(), dtype=np.float32)


def kernel(input, target):
    sx, sy, sxy, _ = _device_sums(input, target)
    return _loss_from_sums(sx, sy, sxy)


if __name__ == "__main__":
    rng = np.random.default_rng(0)
    x = rng.random((B, 1, H, W), dtype=np.float32)
    y = rng.random((B, 1, H, W), dtype=np.float32)
    got = kernel(input=x, target=y)
    xf = x.reshape(B, -1).astype(np.float64)
    yf = y.reshape(B, -1).astype(np.float64)
    det = HW * (xf * yf).sum(1) - xf.sum(1) * yf.sum(1)
    want = (-np.log(np.abs(det) + DET_EPS)).mean()
    print("kernel:", got, "numpy:", want, "rel:", abs(got - want) / abs(want))
